# revision 3
# baseline (speedup 1.0000x reference)
"""CfC Liquid Cell kernel for Trainium2 (Bass/Tile), 8 NeuronCores.

Sharding: data-parallel over batch (B=8 -> 1 batch element per core).
Per-core plan (S=2048, H=1024, NH=16, HD=64, NS=64, K=4):

Phase A (chunks of TA=512 over time):
  - DMA x chunk (time-major), PE-transpose to feature-major x^T
  - in_proj matmuls (weights stationary, x^T moving) -> xz^T in PSUM
  - x_path half copied to SBUF, z half silu'd; both stored to DRAM scratch

Phase B (chunks of TB=256 over time):
  - depthwise causal conv = 4 shifted diagonal matmuls + bias "tap" on PE,
    accumulated in PSUM; silu on ACT
  - head matmuls (bb/f1/f2/tau/decay/state_out) with 2-head block-diagonal
    64x64 weights -> full 128-partition tiles
  - all activations via tanh/silu only (single ACT table set):
    sigmoid(u) = 0.5 + 0.5*tanh(u/2)
  - h_t = d_t*h_{t-1} + (1-d_t)*c_t via DVE tensor_tensor_scan
  - out_proj with gated activations as the *stationary* operand -> y is
    produced time-major, DMA'd straight out (no output transpose)
"""

import numpy as np

B, S, H = 8, 2048, 1024
NH, HD, NS, K = 16, 64, 64, 4
N_CORES = 8
TA = 512            # phase A time chunk
TB = 256            # phase B time chunk
NA = S // TA        # 4
NB = S // TB        # 8
P = 128

_CACHE = {}


def _build_program():
    import concourse.bacc as bacc
    import concourse.mybir as mybir
    import concourse.tile as tile

    F32 = mybir.dt.float32
    AF = mybir.ActivationFunctionType
    ALU = mybir.AluOpType

    nc = bacc.Bacc("TRN2", target_bir_lowering=False, debug=False)

    x_d = nc.dram_tensor("x", (S, H), F32, kind="ExternalInput").ap()
    w_in_d = nc.dram_tensor("w_in", (P, 8, 2 * H), F32, kind="ExternalInput").ap()
    w_out_d = nc.dram_tensor("w_out", (P, 8, H), F32, kind="ExternalInput").ap()
    cdiag_d = nc.dram_tensor("cdiag", (P, 8, 5, P), F32, kind="ExternalInput").ap()
    blk_d = nc.dram_tensor("blk", (P, 6, P), F32, kind="ExternalInput").ap()
    bias_d = nc.dram_tensor("bias", (P, 6), F32, kind="ExternalInput").ap()
    ident_d = nc.dram_tensor("ident", (P, P), F32, kind="ExternalInput").ap()
    y_d = nc.dram_tensor("y", (S, H), F32, kind="ExternalOutput").ap()

    with tile.TileContext(nc) as tc:
        # DRAM scratch for the phase A -> phase B handoff (feature-major)
        with tc.tile_pool(name="dram", bufs=1, space="DRAM") as dpool:
            xp_d = dpool.tile([8, P, S], F32)   # x_path^T  (c-tile, partition, time)
            zs_d = dpool.tile([8, P, S], F32)   # silu(z)^T

            # ---------------- Phase A ----------------
            with tc.tile_pool(name="ca", bufs=1) as ca:
                w_in = ca.tile([P, 8, 2 * H], F32)
                nc.sync.dma_start(w_in[:], w_in_d[:])
                ident = ca.tile([P, P], F32)
                nc.sync.dma_start(ident[:], ident_d[:])

                with tc.tile_pool(name="pax", bufs=2) as pax, \
                     tc.tile_pool(name="paxT", bufs=1) as paxT, \
                     tc.tile_pool(name="paout", bufs=2) as paout, \
                     tc.tile_pool(name="patr", bufs=4, space="PSUM") as patr, \
                     tc.tile_pool(name="pamm", bufs=3, space="PSUM") as pamm:
                    for c in range(NA):
                        xc = pax.tile([P, TA // P, H], F32, tag="xc")
                        nc.sync.dma_start(
                            xc[:],
                            x_d[c * TA:(c + 1) * TA, :].rearrange(
                                "(ss p) k -> p ss k", p=P))
                        xT = paxT.tile([P, 8, TA], F32, tag="xT")
                        for ss in range(TA // P):
                            for kt in range(8):
                                pt = patr.tile([P, P], F32, tag="tr")
                                nc.tensor.transpose(
                                    pt[:], xc[:, ss, kt * P:(kt + 1) * P], ident[:])
                                if kt % 2 == 0:
                                    nc.vector.tensor_copy(
                                        xT[:, kt, ss * P:(ss + 1) * P], pt[:])
                                else:
                                    nc.scalar.activation(
                                        xT[:, kt, ss * P:(ss + 1) * P], pt[:], AF.Copy)
                        xp = paout.tile([P, 8, TA], F32, tag="xp")
                        zs = paout.tile([P, 8, TA], F32, tag="zs")
                        for jt in range(16):
                            pm = pamm.tile([P, TA], F32, tag="mm")
                            for kt in range(8):
                                nc.tensor.matmul(
                                    pm[:], w_in[:, kt, jt * P:(jt + 1) * P],
                                    xT[:, kt, :],
                                    start=(kt == 0), stop=(kt == 7))
                            if jt < 8:
                                nc.vector.tensor_copy(xp[:, jt, :], pm[:])
                            else:
                                nc.scalar.activation(zs[:, jt - 8, :], pm[:], AF.Silu)
                        nc.sync.dma_start(
                            xp_d[:, :, c * TA:(c + 1) * TA].transpose([1, 0, 2]), xp[:])
                        nc.sync.dma_start(
                            zs_d[:, :, c * TA:(c + 1) * TA].transpose([1, 0, 2]), zs[:])

            # ---------------- Phase B ----------------
            with tc.tile_pool(name="cb", bufs=1) as cb:
                w_out = cb.tile([P, 8, H], F32)
                nc.sync.dma_start(w_out[:], w_out_d[:])
                cdiag = cb.tile([P, 8, 5, P], F32)
                nc.sync.dma_start(cdiag[:], cdiag_d[:])
                blk = cb.tile([P, 6, P], F32)
                nc.sync.dma_start(blk[:], blk_d[:])
                bias = cb.tile([P, 6], F32)
                nc.sync.dma_start(bias[:], bias_d[:])
                ones = cb.tile([P, TB], F32)
                nc.vector.memset(ones[:], 1.0)

                with tc.tile_pool(name="pbs", bufs=2) as pbs, \
                     tc.tile_pool(name="work", bufs=10) as work, \
                     tc.tile_pool(name="ph", bufs=2) as ph, \
                     tc.tile_pool(name="py", bufs=2) as pyp, \
                     tc.tile_pool(name="psst", bufs=3, space="PSUM") as psst, \
                     tc.tile_pool(name="psy", bufs=1, space="PSUM") as psy:
                    h_prev = None
                    for c in range(NB):
                        xpc = pbs.tile([P, 8, 3 + TB], F32, tag="xpc")
                        if c == 0:
                            nc.vector.memset(xpc[:, :, :3], 0.0)
                            nc.sync.dma_start(
                                xpc[:, :, 3:],
                                xp_d[:, :, 0:TB].transpose([1, 0, 2]))
                        else:
                            nc.sync.dma_start(
                                xpc[:],
                                xp_d[:, :, c * TB - 3:(c + 1) * TB].transpose([1, 0, 2]))
                        zc = pbs.tile([P, 8, TB], F32, tag="zc", bufs=1)
                        nc.sync.dma_start(
                            zc[:], zs_d[:, :, c * TB:(c + 1) * TB].transpose([1, 0, 2]))

                        # conv (4 shifted diag taps + bias tap) -> silu -> xh
                        xh = work.tile([P, 8, TB], F32, tag="work")
                        for hf in range(2):
                            pc = psst.tile([P, 4, TB], F32, tag="stage")
                            for c4 in range(4):
                                ct = hf * 4 + c4
                                for tap in range(4):
                                    nc.tensor.matmul(
                                        pc[:, c4, :], cdiag[:, ct, tap, :],
                                        xpc[:, ct, tap:tap + TB],
                                        start=(tap == 0), stop=False)
                                nc.tensor.matmul(
                                    pc[:, c4, :], cdiag[:, ct, 4, :], ones[:],
                                    start=False, stop=True)
                            nc.scalar.activation(
                                xh[:, hf * 4:(hf + 1) * 4, :], pc[:], AF.Silu)

                        # backbone: bb = silu(xh @ bb_w + bb_b)
                        bb = work.tile([P, 8, TB], F32, tag="work")
                        for hf in range(2):
                            pb = psst.tile([P, 4, TB], F32, tag="stage")
                            for p4 in range(4):
                                nc.tensor.matmul(
                                    pb[:, p4, :], blk[:, 0, :], xh[:, hf * 4 + p4, :],
                                    start=True, stop=True)
                            nc.scalar.activation(
                                bb[:, hf * 4:(hf + 1) * 4, :], pb[:], AF.Silu,
                                bias=bias[:, 0:1])

                        # gates: f1, f2 (tanh), t_tau, t_d (tanh of u/2)
                        # tau/decay: the 0.5 of tanh(u/2) is folded into the
                        # block-diagonal weights and biases host-side, so ACT
                        # scale stays 1.0 for every gate.
                        gate_specs = [
                            (1, 1.0, "f1"), (2, 1.0, "f2"),
                            (3, 1.0, "ttau"), (4, 1.0, "td"),
                        ]
                        gout = {}
                        for widx, scale, gname in gate_specs:
                            gt = work.tile([P, 8, TB], F32, tag="work", name=gname)
                            for hf in range(2):
                                pg = psst.tile([P, 4, TB], F32, tag="stage")
                                for p4 in range(4):
                                    nc.tensor.matmul(
                                        pg[:, p4, :], blk[:, widx, :],
                                        bb[:, hf * 4 + p4, :],
                                        start=True, stop=True)
                                nc.scalar.activation(
                                    gt[:, hf * 4:(hf + 1) * 4, :], pg[:], AF.Tanh,
                                    bias=bias[:, widx:widx + 1], scale=scale)
                            gout[gname] = gt

                        f1, f2, ttau, td = gout["f1"], gout["f2"], gout["ttau"], gout["td"]
                        tau = work.tile([P, 8, TB], F32, tag="work")
                        nc.vector.tensor_scalar(tau[:], ttau[:], 0.5, 0.5, ALU.mult, ALU.add)
                        dd = work.tile([P, 8, TB], F32, tag="work")
                        nc.vector.tensor_scalar(dd[:], td[:], 0.5, 0.5, ALU.mult, ALU.add)
                        dneg = work.tile([P, 8, TB], F32, tag="work")
                        nc.vector.tensor_scalar(dneg[:], td[:], -0.5, 0.5, ALU.mult, ALU.add)
                        delta = work.tile([P, 8, TB], F32, tag="work")
                        nc.gpsimd.tensor_tensor(delta[:], f2[:], f1[:], ALU.subtract)
                        m = work.tile([P, 8, TB], F32, tag="work")
                        nc.vector.tensor_tensor(m[:], delta[:], tau[:], ALU.mult)
                        nc.vector.tensor_tensor(m[:], m[:], f1[:], ALU.add)
                        cp = work.tile([P, 8, TB], F32, tag="work")
                        nc.vector.tensor_tensor(cp[:], m[:], dneg[:], ALU.mult)

                        # scan: h_t = d_t*h_{t-1} + cp_t  (per 128-lane tile)
                        h = ph.tile([P, 8, TB], F32, tag="h")
                        for lt in range(8):
                            init = 0.0 if c == 0 else h_prev[:, lt, TB - 1:TB]
                            nc.vector.tensor_tensor_scan(
                                h[:, lt, :], dd[:, lt, :], cp[:, lt, :], init,
                                ALU.mult, ALU.add)
                        h_prev = h

                        # state out: oseq = h @ so_w + so_b
                        oseq = work.tile([P, 8, TB], F32, tag="work")
                        for hf in range(2):
                            po = psst.tile([P, 4, TB], F32, tag="stage")
                            for p4 in range(4):
                                nc.tensor.matmul(
                                    po[:, p4, :], blk[:, 5, :], h[:, hf * 4 + p4, :],
                                    start=True, stop=True)
                            nc.scalar.activation(
                                oseq[:, hf * 4:(hf + 1) * 4, :], po[:], AF.Identity,
                                bias=bias[:, 5:6])

                        # gating: g = oseq * silu(z)
                        g = work.tile([P, 8, TB], F32, tag="work")
                        nc.gpsimd.tensor_tensor(g[:], oseq[:], zc[:], ALU.mult)

                        # out_proj: y (time-major) = g^T.T @ w_out
                        ysb = pyp.tile([P, TB // P, H], F32, tag="y")
                        for st in range(TB // P):
                            py = psy.tile([P, H], F32, tag="ypsum")
                            for kt in range(8):
                                lh = g[:, kt, st * P:(st + 1) * P]
                                nc.tensor.matmul(
                                    py[:, 0:512], lh, w_out[:, kt, 0:512],
                                    start=(kt == 0), stop=(kt == 7))
                                nc.tensor.matmul(
                                    py[:, 512:1024], lh, w_out[:, kt, 512:1024],
                                    start=(kt == 0), stop=(kt == 7))
                            nc.vector.tensor_copy(ysb[:, st, :], py[:])
                        nc.sync.dma_start(
                            y_d[c * TB:(c + 1) * TB, :].rearrange(
                                "(st p) j -> p st j", p=P),
                            ysb[:])

    nc.compile()
    return nc


def _prep_shared(inputs):
    """Host-side preprocessing of the shared (weight) tensors."""
    f32 = np.float32
    in_proj_w = np.asarray(inputs["in_proj_w"], f32)
    conv_w = np.asarray(inputs["conv_w"], f32)
    conv_b = np.asarray(inputs["conv_b"], f32)

    w_in = in_proj_w.reshape(8, P, 2 * H).transpose(1, 0, 2).copy()
    w_out = np.asarray(inputs["out_proj_w"], f32).reshape(8, P, H).transpose(1, 0, 2).copy()

    cdiag = np.zeros((8, 5, P, P), f32)
    rng = np.arange(P)
    for ct in range(8):
        for tap in range(K):
            cdiag[ct, tap, rng, rng] = conv_w[ct * P:(ct + 1) * P, 0, tap]
        cdiag[ct, 4, rng, rng] = conv_b[ct * P:(ct + 1) * P]
    cdiag = cdiag.transpose(2, 0, 1, 3).copy()  # (P, 8, 5, P)

    def blk2(w):
        o = np.zeros((P, P), f32)
        o[:64, :64] = w
        o[64:, 64:] = w
        return o

    blk = np.stack([
        blk2(np.asarray(inputs["bb_w"], f32)),
        blk2(np.asarray(inputs["f1_w"], f32)),
        blk2(np.asarray(inputs["f2_w"], f32)),
        blk2(np.asarray(inputs["tau_a_w"], f32) * 0.5),
        blk2(np.asarray(inputs["decay_w"], f32) * 0.5),
        blk2(np.asarray(inputs["state_out_w"], f32)),
    ], axis=1)  # (P, 6, P)

    def t2(v):
        return np.tile(np.asarray(v, f32), 2)

    bias = np.stack([
        t2(inputs["bb_b"]),
        t2(inputs["f1_b"]),
        t2(inputs["f2_b"]),
        0.5 * (t2(inputs["tau_a_b"]) + t2(inputs["tau_b"])),
        0.5 * t2(inputs["decay_b"]),
        t2(inputs["state_out_b"]),
    ], axis=1)  # (P, 6)

    ident = np.eye(P, dtype=f32)
    return {
        "w_in": np.ascontiguousarray(w_in),
        "w_out": np.ascontiguousarray(w_out),
        "cdiag": np.ascontiguousarray(cdiag),
        "blk": np.ascontiguousarray(blk),
        "bias": np.ascontiguousarray(bias),
        "ident": ident,
    }


def kernel(**inputs) -> np.ndarray:
    from concourse import bass_utils

    if "nc" not in _CACHE:
        _CACHE["nc"] = _build_program()
    nc = _CACHE["nc"]

    shared = _prep_shared(inputs)
    x = np.asarray(inputs["x"], np.float32)

    in_maps = []
    for b in range(N_CORES):
        m = dict(shared)
        m["x"] = np.ascontiguousarray(x[b])
        in_maps.append(m)

    res = bass_utils.run_bass_kernel_spmd(nc, in_maps, core_ids=list(range(N_CORES)))
    out = np.stack([res.results[b]["y"] for b in range(N_CORES)], axis=0)
    return out.astype(np.float32)


# NOTE on tau/decay: the reference computes
#   tau   = sigmoid(bb @ tau_a_w + tau_a_b + tau_b)
#   decay = sigmoid(bb @ decay_w + decay_b)
# We use sigmoid(u) = 0.5 + 0.5*tanh(u/2): the 0.5 on u is folded into the
# block-diagonal weights (tau_a_w*0.5, decay_w*0.5) and the biases
# (0.5*(tau_a_b+tau_b), 0.5*decay_b); ACT computes tanh(psum + bias) with
# scale 1.0, and the DVE affine 0.5*t + 0.5 recovers the sigmoid.
